# revision 7
# baseline (speedup 1.0000x reference)
"""Trainium2 Bass kernel for nn_Attention_59287728554369.

Multi-head cross-attention, b=2, nq=nk=2048, 16 heads x 64 dim, d_model=1024.
Sharding: batch (2) x head-groups (4 heads each) -> 8 cores.
Each core computes q/k/v projections for its 4 heads, fused masked softmax
attention, and a partial output projection; host sums the 4 partials per batch.

v3 (from v2's 162us HW / 178us sim; ACT-exp floor is ~127us/pass):
- mask/padding moved entirely into the V ones-column (vones=keep-flag per
  key): padding keys have zero context rows -> K=V=0 -> exp(0)=1 contributes
  1*0 to numerator and 0 to denominator. Removes the kb bias input and the
  per-jt exp bias coupling.
- attention restructured from per-block loops into ONE global (block, jt)
  stream with the depth-2 st prefetch crossing block boundaries, so the ACT
  exp stream never breaks at hp/iblock transitions.
- pipe mode: pass k emits pass k+1's phase-1 work (q/ct block-0 DMAs +
  qt/kt/v block-0 projections, double-buffered per-set activation tiles) as
  PE filler inside its last attention blocks, in addition to deferring its
  own final oproj to pass k+1 (v2 mechanism). ACT then flows between passes
  with no projection preamble bubble.
- vones DMA hoisted out of the pass loop (written once per buffer set).
- bf16 everywhere, compacted keys, merged 3D DMAs, denominator via
  ones-augmented V column, fast-reciprocal+gpsimd-broadcast normalize (v2).
"""
import os
import sys

sys.path.insert(0, "/opt/trn_rl_repo")

import numpy as np

import concourse.bass as bass  # noqa: F401
import concourse.tile as tile
from concourse import bacc, mybir

F32 = mybir.dt.float32
BF16 = mybir.dt.bfloat16
AF = mybir.ActivationFunctionType

# Problem constants (hardcoded per contest rules)
B = 2
NQ = 2048
NK = 2048
D = 1024          # d_model
H = 16            # total heads
DH = 64           # head dim
HG = 4            # heads per core
CG = HG * DH      # channels per core = 256
N_CORES = 8
SCALE = DH ** -0.5
# V per-head stride: 64 data + 1 ones + 3 pad so each head slice starts
# 8-byte aligned in bf16 (65*2=130B bases mis-address PE ldweights on HW)
VSTR = 68

_CACHE = {}


def build_nc(reps=1, nkc=NK):
    """Build the single-core Bass program (identical across cores).

    nkc: compacted key count (multiple of 128, <= NK).
    reps>1 wraps the computation in an on-device For_i loop (same buffers) so
    test harnesses can measure marginal wall time per rep = HW exec time.
    """
    assert nkc % 128 == 0 and 128 <= nkc <= NK
    JTC = nkc // 128               # 128-wide j tiles
    # j blocks for the projections: full 512s plus one remainder block
    jblocks = [(s, 512) for s in range(0, nkc - nkc % 512, 512)]
    if nkc % 512:
        jblocks.append((nkc - nkc % 512, nkc % 512))

    nc = bacc.Bacc("TRN2", target_bir_lowering=False, debug=False)

    qT = nc.dram_tensor("qT", [D, NQ], BF16, kind="ExternalInput").ap()
    cT = nc.dram_tensor("cT", [D, nkc], BF16, kind="ExternalInput").ap()
    wq = nc.dram_tensor("wq", [D, CG], BF16, kind="ExternalInput").ap()
    wk = nc.dram_tensor("wk", [D, CG], BF16, kind="ExternalInput").ap()
    wv = nc.dram_tensor("wv", [D, CG], BF16, kind="ExternalInput").ap()
    wo = nc.dram_tensor("wo", [CG, D], BF16, kind="ExternalInput").ap()
    vones = nc.dram_tensor("vones", [128, JTC * HG * 4], BF16, kind="ExternalInput").ap()
    outp = nc.dram_tensor("outp", [NQ, D], BF16, kind="ExternalOutput").ap()

    KT = 8   # k tiles over d_model
    IB = 4   # 512-wide i blocks

    with tile.TileContext(nc) as tc:
        with tc.tile_pool(name="sb", bufs=1) as sb:
            # ---- persistent SBUF tensors; DMA issue order puts the first
            # projection's dependencies (wq, wk, qT block 0, cT block 0)
            # at the head of the queue ----
            wq_sb = sb.tile([128, KT, CG], BF16, bufs=1)
            nc.sync.dma_start(out=wq_sb, in_=wq.rearrange("(t p) c -> p t c", p=128))
            wk_sb = sb.tile([128, KT, CG], BF16, bufs=1)
            nc.sync.dma_start(out=wk_sb, in_=wk.rearrange("(t p) c -> p t c", p=128))

            def _qt_dma(ib2):
                a = sb.tile([128, KT, 512], BF16, tag="act", bufs=5, name="act")
                nc.sync.dma_start(
                    out=a,
                    in_=qT.rearrange("(t p) i -> p t i", p=128)[
                        :, :, ib2 * 512:(ib2 + 1) * 512],
                )
                return [a[:, k, :] for k in range(KT)]

            def _ct_dma(j0, bw):
                a = sb.tile([128, KT, 512], BF16, tag="act", bufs=5, name="act")
                nc.sync.dma_start(
                    out=a[:, :, 0:bw],
                    in_=cT.rearrange("(t p) j -> p t j", p=128)[:, :, j0:j0 + bw],
                )
                return [a[:, k, :] for k in range(KT)]

            # phase-1 block-0 inputs queued before the remaining weights so
            # the first projection matmul starts as early as possible
            p1_qx = _qt_dma(0)
            p1_ct = _ct_dma(jblocks[0][0], jblocks[0][1])

            wv_sb = sb.tile([128, KT, CG], BF16, bufs=1)
            nc.sync.dma_start(out=wv_sb, in_=wv.rearrange("(t p) c -> p t c", p=128))
            wo_sb = sb.tile([128, 2, D], BF16, bufs=1)
            nc.sync.dma_start(out=wo_sb, in_=wo.rearrange("(t p) m -> p t m", p=128))

            pipe = bool(int(os.environ.get("BASS_ATTN_PIPE", "1"))) and reps > 1
            NSETS = 2 if pipe else 1
            # per-set persistent activation tensors (pipe: double-buffered so
            # pass k's tail can project pass k+1's block-0 data)
            g_kt = [[sb.tile([128, nkc], BF16, bufs=1, name=f"kt{s}{i}")
                     for i in range(2)] for s in range(NSETS)]
            g_qt = [[sb.tile([128, NQ], BF16, bufs=1, name=f"qt{s}{i}")
                     for i in range(2)] for s in range(NSETS)]
            g_v = [sb.tile([128, JTC, HG * VSTR], BF16, bufs=1, name=f"v{s}")
                   for s in range(NSETS)]
            g_ot = [[sb.tile([128, NQ], BF16, bufs=1, name=f"ot{s}{i}")
                     for i in range(2)] for s in range(NSETS)]
            for s in range(NSETS):
                nc.sync.dma_start(
                    out=g_v[s].rearrange("p t (h e) -> p t h e", e=VSTR)[:, :, :, 64:VSTR],
                    in_=vones.rearrange("p (t h e) -> p t h e", h=HG, e=4),
                )

            with tc.tile_pool(name="ps", bufs=1, space="PSUM") as ps:

                def _qt_mm(s, ib2, qx_t, cb):
                    qt_ps = ps.tile([128, 512], F32, tag="mm", bufs=2, name="qt_ps")
                    for k in range(KT):
                        nc.tensor.matmul(
                            qt_ps,
                            wq_sb[:, k, cb * 128:(cb + 1) * 128],
                            qx_t[k],
                            start=(k == 0),
                            stop=(k == KT - 1),
                        )
                    nc.vector.tensor_copy(
                        g_qt[s][cb][:, ib2 * 512:(ib2 + 1) * 512], qt_ps
                    )

                def _kproj_mm(s, j0, bw, ct_t, cb):
                    kt_ps = ps.tile([128, 512], F32, tag="mm", bufs=2, name="kt_ps")
                    for k in range(KT):
                        nc.tensor.matmul(
                            kt_ps[:, 0:bw],
                            wk_sb[:, k, cb * 128:(cb + 1) * 128],
                            ct_t[k][:, 0:bw],
                            start=(k == 0),
                            stop=(k == KT - 1),
                        )
                    nc.vector.tensor_copy(g_kt[s][cb][:, j0:j0 + bw], kt_ps[:, 0:bw])

                def _vproj_mm(s, j0, ct_t, js):
                    # tag "mm", not "pv": when run as filler inside the
                    # attention stream both pv bufs are held by the open
                    # PV accumulators (tag-pv alloc would deadlock)
                    v_ps = ps.tile([128, CG], F32, tag="mm", bufs=2, name="v_ps")
                    for k in range(KT):
                        nc.tensor.matmul(
                            v_ps,
                            ct_t[k][:, js * 128:(js + 1) * 128],
                            wv_sb[:, k, :],
                            start=(k == 0),
                            stop=(k == KT - 1),
                        )
                    nc.vector.tensor_copy(
                        g_v[s][:, j0 // 128 + js].rearrange(
                            "p (h e) -> p h e", e=VSTR
                        )[:, :, 0:64],
                        v_ps.rearrange("p (h e) -> p h e", e=64),
                    )

                def _phase1_inline(s, qx0=None, ct_t=None):
                    j0, bw = jblocks[0]
                    if qx0 is None:
                        qx0 = _qt_dma(0)
                        ct_t = _ct_dma(j0, bw)
                    _qt_mm(s, 0, qx0, 0)
                    _qt_mm(s, 0, qx0, 1)
                    for cb in range(2):
                        _kproj_mm(s, j0, bw, ct_t, cb)
                    for js in range(bw // 128):
                        _vproj_mm(s, j0, ct_t, js)

                def _emit_st(s, hp, ib2, jt):
                    st = ps.tile([128, 1024], F32, tag="st", bufs=2, name="st")
                    for b in range(2):
                        nc.tensor.matmul(
                            st[:, b * 512:(b + 1) * 512],
                            g_kt[s][hp][b * 64:(b + 1) * 64, jt * 128:(jt + 1) * 128],
                            g_qt[s][hp][b * 64:(b + 1) * 64, ib2 * 512:(ib2 + 1) * 512],
                            start=True,
                            stop=True,
                        )
                    return st

                def _exp(st):
                    e = sb.tile([128, 1024], BF16, tag="et", bufs=4, name="e")
                    nc.scalar.activation(e, st, AF.Exp, scale=SCALE)
                    return e

                def _alloc_pvs():
                    return [
                        ps.tile([65, 512], F32, tag="pv", bufs=2, name="pv")
                        for _ in range(2)
                    ]

                def _pv_step(s, hp, jt, e, pvs):
                    for b in range(2):
                        h = 2 * hp + b
                        nc.tensor.matmul(
                            pvs[b],
                            g_v[s][:, jt, h * VSTR:h * VSTR + 65],
                            e[:, b * 512:(b + 1) * 512],
                            start=(jt == 0),
                            stop=(jt == JTC - 1),
                        )

                def _normalize(s, hp, ib2, pvs, tail=False):
                    """tail=True shortens the chain-to-ot latency for the
                    final block (reps=1 path): denominator chain first and
                    the big PSUM->SBUF copies on the (by then idle) ACT
                    engine. Otherwise prioritize freeing the pv PSUM banks.

                    (dr copy: reciprocal_approx_fast mis-addresses on HW
                    when input/output base partitions differ, so the
                    denominator row is copied to partition 0 first.)"""
                    pvcs, dens = [], []
                    if not tail:
                        for b in range(2):
                            pvc = sb.tile([65, 512], F32, tag="pvc", bufs=2, name="pvc")
                            nc.vector.tensor_copy(pvc, pvs[b])
                            pvcs.append(pvc)
                    for b in range(2):
                        dr = sb.tile([1, 512], F32, tag="dr", bufs=2, name="dr")
                        nc.vector.tensor_copy(dr, pvs[b][64:65, :])
                        rec = sb.tile([1, 512], F32, tag="rec", bufs=2, name="rec")
                        nc.vector.reciprocal_approx_fast(out=rec, in_=dr)
                        den = sb.tile([64, 512], F32, tag="den", bufs=2, name="den")
                        nc.gpsimd.partition_broadcast(den, rec[0:1, :])
                        dens.append(den)
                    if tail:
                        for b in range(2):
                            pvc = sb.tile([65, 512], F32, tag="pvc", bufs=2, name="pvc")
                            nc.scalar.activation(pvc, pvs[b], AF.Copy)
                            pvcs.append(pvc)
                    for b in range(2):
                        nc.vector.tensor_mul(
                            g_ot[s][hp][b * 64:(b + 1) * 64,
                                        ib2 * 512:(ib2 + 1) * 512],
                            pvcs[b][0:64, :],
                            dens[b],
                        )

                def _oproj_unit(s, ib2, it, m, copy_act=False, ot=None):
                    ot = ot if ot is not None else g_ot[s]
                    itg = ib2 * 4 + it
                    op = ps.tile([128, 512], F32, tag="mm", bufs=2, name="op")
                    for kk in range(2):
                        nc.tensor.matmul(
                            op,
                            ot[kk][:, itg * 128:(itg + 1) * 128],
                            wo_sb[:, kk, m * 512:(m + 1) * 512],
                            start=(kk == 0),
                            stop=(kk == 1),
                        )
                    osb = sb.tile([128, 512], BF16, tag="osb", bufs=3, name="osb")
                    if copy_act:
                        nc.scalar.activation(osb, op, AF.Copy)
                    else:
                        nc.vector.tensor_copy(osb, op)
                    nc.sync.dma_start(
                        out=outp[itg * 128:(itg + 1) * 128, m * 512:(m + 1) * 512],
                        in_=osb,
                    )

                def _oproj_fillers(s, ib2, units, jt0):
                    # each unit is 2 matmuls; jt0 leaves room for the
                    # producing normalize chain to finish
                    return [
                        (jt0 + i,
                         (lambda ib2=ib2, it=it, m=m: _oproj_unit(s, ib2, it, m)))
                        for i, (it, m) in enumerate(
                            ((u // 2, u % 2) for u in units)
                        )
                    ]

                def _qtproj_fillers(s, ib2):
                    # DMA issue up front; the 2x8 matmuls from jt=6
                    box = {}

                    def dma(ib2=ib2):
                        box["qx"] = _qt_dma(ib2)

                    return [(0, dma)] + [
                        (6 + 3 * cb,
                         (lambda ib2=ib2, cb=cb: _qt_mm(s, ib2, box["qx"], cb)))
                        for cb in range(2)
                    ]

                def _kv_fillers(s, blocks):
                    """K/V projection for jblocks[1:], interleaved into the
                    first attention block: block k (j-tiles 4k..4k+3) is
                    emitted by jt 4(k-1) so the ST prefetch (jt+2) never
                    outruns the kt/v writes."""
                    out = []
                    for k, (j0, bw) in blocks:
                        base = 4 * (k - 1)
                        box = {}

                        def dma(j0=j0, bw=bw, box=box):
                            box["ct"] = _ct_dma(j0, bw)

                        out.append((base, dma))
                        for cb in range(2):
                            out.append(
                                (base + cb, lambda j0=j0, bw=bw, cb=cb, box=box:
                                 _kproj_mm(s, j0, bw, box["ct"], cb))
                            )
                        for js in range(bw // 128):
                            out.append(
                                (base + 2 + js // 2, lambda j0=j0, js=js, box=box:
                                 _vproj_mm(s, j0, box["ct"], js))
                            )
                    return out

                def _phase1_fillers(s):
                    """Pass k's tail fillers computing pass k+1's (set s)
                    block-0 data. Returned as {block_idx: [(jt, fn), ...]}
                    for the last four attention blocks (idx 4..7)."""
                    j0, bw = jblocks[0]
                    box = {}

                    def qdma():
                        box["qx"] = _qt_dma(0)

                    def cdma(j0=j0, bw=bw):
                        box["ct"] = _ct_dma(j0, bw)

                    return {
                        3: [(9, qdma), (10, cdma)],
                        4: [(6, lambda: _qt_mm(s, 0, box["qx"], 0)),
                            (9, lambda: _qt_mm(s, 0, box["qx"], 1))],
                        5: [(6, lambda: _kproj_mm(s, j0, bw, box["ct"], 0)),
                            (9, lambda: _kproj_mm(s, j0, bw, box["ct"], 1))],
                        6: [(2 + 3 * js, lambda js=js: _vproj_mm(s, j0, box["ct"], js))
                            for js in range(bw // 128)],
                    }

                def _one_pass(s, defer_in=None, prep_next=None, tail_normalize=False):
                    """One full pass using activation-tile set s.

                    defer_in: the OTHER set's ot pair -- emit its final-block
                    oproj units here (their data has been ready since the
                    previous pass), and skip our own final-block oproj.
                    prep_next: set index to run phase-1 for, as tail fillers.
                    """
                    # per-block filler queues, keyed by block index in the
                    # global stream order [(hp0,ib0),(hp1,ib0),(hp0,ib1),...]
                    fq = {bi: [] for bi in range(2 * IB)}
                    fq[0] = _kv_fillers(s, list(enumerate(jblocks))[1:])
                    for ib2 in range(1, IB):
                        fq[2 * ib2] = _oproj_fillers(s, ib2 - 1, range(0, 6), 4)
                    for ib2 in range(0, IB - 1):
                        fl = list(_qtproj_fillers(s, ib2 + 1))
                        if ib2 >= 1:
                            fl += _oproj_fillers(s, ib2 - 1, range(6, 8), 2)
                        fl.sort(key=lambda x: x[0])
                        fq[2 * ib2 + 1] = fl
                    fq[2 * IB - 1] = _oproj_fillers(s, IB - 2, range(6, 8), 2)
                    if defer_in is not None:
                        dunits = [
                            (lambda u=u: _oproj_unit(
                                s, IB - 1, u // 2, u % 2, ot=defer_in))
                            for u in range(8)
                        ]
                        fq[2 * IB - 2] = sorted(
                            fq[2 * IB - 2] + list(enumerate(dunits[:4])),
                            key=lambda x: x[0])
                        fq[2 * IB - 1] = sorted(
                            fq[2 * IB - 1] + list(enumerate(dunits[4:])),
                            key=lambda x: x[0])
                    if prep_next is not None:
                        for bi, fl in _phase1_fillers(prep_next).items():
                            fq[bi] = sorted(fq[bi] + fl, key=lambda x: x[0])

                    # global attention stream: depth-2 st prefetch crossing
                    # block boundaries so exp never waits at a transition
                    steps = [(2 * ib2 + hp, hp, ib2, jt)
                             for ib2 in range(IB) for hp in range(2)
                             for jt in range(JTC)]
                    st_q = {}
                    for i in range(2):
                        _, hp, ib2, jt = steps[i]
                        st_q[i] = _emit_st(s, hp, ib2, jt)
                    pvs_by_block = {}
                    for idx, (bi, hp, ib2, jt) in enumerate(steps):
                        if jt == 0:
                            pvs_by_block[bi] = _alloc_pvs()
                        e = _exp(st_q.pop(idx))
                        if idx + 2 < len(steps):
                            _, hp2, ib22, jt2 = steps[idx + 2]
                            st_q[idx + 2] = _emit_st(s, hp2, ib22, jt2)
                        _pv_step(s, hp, jt, e, pvs_by_block[bi])
                        q = fq[bi]
                        while q and q[0][0] <= jt:
                            q.pop(0)[1]()
                        if jt == JTC - 1:
                            _normalize(s, hp, ib2, pvs_by_block.pop(bi),
                                       tail=(tail_normalize and bi == 2 * IB - 1))
                            for _, f in fq[bi]:
                                f()
                            fq[bi] = []

                    if defer_in is None:
                        for u in range(8):
                            _oproj_unit(s, IB - 1, u // 2, u % 2,
                                        copy_act=(u % 2 == 1))

                if reps == 1:
                    _phase1_inline(0, p1_qx, p1_ct)
                    _one_pass(0, tail_normalize=True)
                elif pipe:
                    # two unrolled passes per iteration with alternating
                    # buffer sets; each pass emits the OTHER set's final
                    # oproj (previous pass's data; garbage on iteration 0,
                    # harmless for timing-only loops) and the other set's
                    # phase-1 projections for the NEXT pass.
                    assert reps % 2 == 0, "pipelined builds need even reps"
                    _phase1_inline(0, p1_qx, p1_ct)
                    with tc.For_i(0, reps // 2, 1):
                        _one_pass(0, defer_in=g_ot[1], prep_next=1)
                        _one_pass(1, defer_in=g_ot[0], prep_next=0)
                else:
                    with tc.For_i(0, reps, 1):
                        _phase1_inline(0)
                        _one_pass(0, tail_normalize=True)

    nc.compile()
    return nc


def _nkc_for_mask(mask):
    """Compacted key count: max unmasked keys over batches, rounded to 128."""
    counts = [int((~mask[bi]).sum()) for bi in range(mask.shape[0])]
    nkc = max(max(counts), 1)
    nkc = min(((nkc + 127) // 128) * 128, NK)
    return nkc


def _bf16(a):
    import ml_dtypes

    return np.ascontiguousarray(a).astype(ml_dtypes.bfloat16)


def _prep_core_inputs(q, context, mask, Wq, Wkv, Wout, core, nkc=NK):
    bi, g = core // 4, core % 4
    c0 = g * CG
    JTC = nkc // 128
    keep_idx = np.nonzero(~mask[bi])[0]
    ctx_c = np.zeros((nkc, D), dtype=np.float32)
    ctx_c[: len(keep_idx)] = context[bi][keep_idx]
    # per-key keep flag in the V ones-column: padding keys have K=V=0 so
    # their exp(0)=1 weight contributes 0 to both numerator and denominator
    keep = np.zeros(nkc, dtype=np.float32)
    keep[: len(keep_idx)] = 1.0
    km = np.ascontiguousarray(keep.reshape(JTC, 128).T)       # [128, JTC]
    vo = np.zeros((128, JTC, HG, 4), dtype=np.float32)
    vo[:, :, :, 0] = km[:, :, None]
    return {
        "qT": _bf16(q[bi].T),
        "cT": _bf16(ctx_c.T),
        "wq": _bf16(Wq[:, c0:c0 + CG]),
        "wk": _bf16(Wkv[:, c0:c0 + CG]),
        "wv": _bf16(Wkv[:, D + c0:D + c0 + CG]),
        "wo": _bf16(Wout[c0:c0 + CG, :]),
        "vones": _bf16(vo.reshape(128, -1)),
    }


def kernel(q, context, mask, Wq, Wkv, Wout, b_out):
    from concourse.bass_utils import run_bass_kernel_spmd

    q = np.asarray(q, dtype=np.float32)
    context = np.asarray(context, dtype=np.float32)
    mask = np.asarray(mask)
    Wq = np.asarray(Wq, dtype=np.float32)
    Wkv = np.asarray(Wkv, dtype=np.float32)
    Wout = np.asarray(Wout, dtype=np.float32)
    b_out = np.asarray(b_out, dtype=np.float32)

    nkc = _nkc_for_mask(mask)
    key = ("nc", nkc)
    if key not in _CACHE:
        _CACHE[key] = build_nc(nkc=nkc)
    nc = _CACHE[key]
    _CACHE["nc"] = nc
    _CACHE["nkc"] = nkc

    in_maps = [
        _prep_core_inputs(q, context, mask, Wq, Wkv, Wout, c, nkc=nkc)
        for c in range(N_CORES)
    ]

    trace = bool(int(os.environ.get("BASS_ATTN_TRACE", "0")))
    res = run_bass_kernel_spmd(nc, in_maps, list(range(N_CORES)), trace=trace)
    _CACHE["last_results"] = res
    _CACHE["last_in_maps"] = in_maps

    out = np.empty((B, NQ, D), dtype=np.float32)
    for bi in range(B):
        acc = res.results[4 * bi]["outp"].astype(np.float32).copy()
        for g in range(1, 4):
            acc += res.results[4 * bi + g]["outp"].astype(np.float32)
        out[bi] = acc + b_out[None, :]
    return out


# revision 8
# speedup vs baseline: 1.2791x; 1.2791x over previous
"""Trainium2 Bass kernel for nn_Attention_59287728554369.

Multi-head cross-attention, b=2, nq=nk=2048, 16 heads x 64 dim, d_model=1024.
Sharding: batch (2) x head-groups (4 heads each) -> 8 cores.
Each core computes q/k/v projections for its 4 heads, fused masked softmax
attention, and a partial output projection; host sums the 4 partials per batch.

v3 (from v2's 162us HW / 178us sim; ACT-exp floor is ~127us/pass):
- mask/padding moved entirely into the V ones-column (vones=keep-flag per
  key): padding keys have zero context rows -> K=V=0 -> exp(0)=1 contributes
  1*0 to numerator and 0 to denominator. Removes the kb bias input and the
  per-jt exp bias coupling.
- attention restructured from per-block loops into ONE global (block, jt)
  stream with the depth-2 st prefetch crossing block boundaries, so the ACT
  exp stream never breaks at hp/iblock transitions.
- pipe mode: pass k emits pass k+1's phase-1 work (q/ct block-0 DMAs +
  qt/kt/v block-0 projections, double-buffered per-set activation tiles) as
  PE filler inside its last attention blocks, in addition to deferring its
  own final oproj to pass k+1 (v2 mechanism). ACT then flows between passes
  with no projection preamble bubble.
- vones DMA hoisted out of the pass loop (written once per buffer set).
- bf16 everywhere, compacted keys, merged 3D DMAs, denominator via
  ones-augmented V column, fast-reciprocal+gpsimd-broadcast normalize (v2).
"""
import os
import sys

sys.path.insert(0, "/opt/trn_rl_repo")

import numpy as np

import concourse.bass as bass  # noqa: F401
import concourse.tile as tile
from concourse import bacc, mybir

F32 = mybir.dt.float32
BF16 = mybir.dt.bfloat16
AF = mybir.ActivationFunctionType

# Problem constants (hardcoded per contest rules)
B = 2
NQ = 2048
NK = 2048
D = 1024          # d_model
H = 16            # total heads
DH = 64           # head dim
HG = 4            # heads per core
CG = HG * DH      # channels per core = 256
N_CORES = 8
SCALE = DH ** -0.5
# V per-head stride: 64 data + 1 ones + 3 pad so each head slice starts
# 8-byte aligned in bf16 (65*2=130B bases mis-address PE ldweights on HW)
VSTR = 68

_CACHE = {}


def build_nc(reps=1, nkc=NK):
    """Build the single-core Bass program (identical across cores).

    nkc: compacted key count (multiple of 128, <= NK).
    reps>1 wraps the computation in an on-device For_i loop (same buffers) so
    test harnesses can measure marginal wall time per rep = HW exec time.
    """
    assert nkc % 128 == 0 and 128 <= nkc <= NK
    JTC = nkc // 128               # 128-wide j tiles
    # j blocks for the projections: full 512s plus one remainder block
    jblocks = [(s, 512) for s in range(0, nkc - nkc % 512, 512)]
    if nkc % 512:
        jblocks.append((nkc - nkc % 512, nkc % 512))

    nc = bacc.Bacc("TRN2", target_bir_lowering=False, debug=False)

    qT = nc.dram_tensor("qT", [D, NQ], BF16, kind="ExternalInput").ap()
    cT = nc.dram_tensor("cT", [D, nkc], BF16, kind="ExternalInput").ap()
    wq = nc.dram_tensor("wq", [D, CG], BF16, kind="ExternalInput").ap()
    wk = nc.dram_tensor("wk", [D, CG], BF16, kind="ExternalInput").ap()
    wv = nc.dram_tensor("wv", [D, CG], BF16, kind="ExternalInput").ap()
    wo = nc.dram_tensor("wo", [CG, D], BF16, kind="ExternalInput").ap()
    vones = nc.dram_tensor("vones", [128, JTC * HG * 4], BF16, kind="ExternalInput").ap()
    outp = nc.dram_tensor("outp", [NQ, D], BF16, kind="ExternalOutput").ap()

    KT = 8   # k tiles over d_model
    IB = 4   # 512-wide i blocks

    with tile.TileContext(nc) as tc:
        with tc.tile_pool(name="sb", bufs=1) as sb:
            # ---- persistent SBUF tensors; DMA issue order puts the first
            # projection's dependencies (wq, wk, qT block 0, cT block 0)
            # at the head of the queue ----
            wq_sb = sb.tile([128, KT, CG], BF16, bufs=1)
            nc.sync.dma_start(out=wq_sb, in_=wq.rearrange("(t p) c -> p t c", p=128))
            wk_sb = sb.tile([128, KT, CG], BF16, bufs=1)
            nc.sync.dma_start(out=wk_sb, in_=wk.rearrange("(t p) c -> p t c", p=128))

            def _qt_dma(ib2):
                a = sb.tile([128, KT, 512], BF16, tag="act", bufs=5, name="act")
                nc.sync.dma_start(
                    out=a,
                    in_=qT.rearrange("(t p) i -> p t i", p=128)[
                        :, :, ib2 * 512:(ib2 + 1) * 512],
                )
                return [a[:, k, :] for k in range(KT)]

            def _ct_dma(j0, bw):
                a = sb.tile([128, KT, 512], BF16, tag="act", bufs=5, name="act")
                nc.sync.dma_start(
                    out=a[:, :, 0:bw],
                    in_=cT.rearrange("(t p) j -> p t j", p=128)[:, :, j0:j0 + bw],
                )
                return [a[:, k, :] for k in range(KT)]

            # phase-1 block-0 inputs queued before the remaining weights so
            # the first projection matmul starts as early as possible
            p1_qx = _qt_dma(0)
            p1_ct = _ct_dma(jblocks[0][0], jblocks[0][1])

            wv_sb = sb.tile([128, KT, CG], BF16, bufs=1)
            nc.sync.dma_start(out=wv_sb, in_=wv.rearrange("(t p) c -> p t c", p=128))
            wo_sb = sb.tile([128, 2, D], BF16, bufs=1)
            nc.sync.dma_start(out=wo_sb, in_=wo.rearrange("(t p) m -> p t m", p=128))

            pipe = bool(int(os.environ.get("BASS_ATTN_PIPE", "1"))) and reps > 1
            NSETS = 2 if pipe else 1
            # per-set persistent activation tensors (pipe: double-buffered so
            # pass k's tail can project pass k+1's block-0 data)
            g_kt = [[sb.tile([128, nkc], BF16, bufs=1, name=f"kt{s}{i}")
                     for i in range(2)] for s in range(NSETS)]
            g_qt = [[sb.tile([128, NQ], BF16, bufs=1, name=f"qt{s}{i}")
                     for i in range(2)] for s in range(NSETS)]
            g_v = [sb.tile([128, JTC, HG * VSTR], BF16, bufs=1, name=f"v{s}")
                   for s in range(NSETS)]
            g_ot = [[sb.tile([128, NQ], BF16, bufs=1, name=f"ot{s}{i}")
                     for i in range(2)] for s in range(NSETS)]
            for s in range(NSETS):
                nc.sync.dma_start(
                    out=g_v[s].rearrange("p t (h e) -> p t h e", e=VSTR)[:, :, :, 64:VSTR],
                    in_=vones.rearrange("p (t h e) -> p t h e", h=HG, e=4),
                )

            with tc.tile_pool(name="ps", bufs=1, space="PSUM") as ps:

                def _qt_mm(s, ib2, qx_t, cb):
                    qt_ps = ps.tile([128, 512], F32, tag="mm", bufs=2, name="qt_ps")
                    for k in range(KT):
                        nc.tensor.matmul(
                            qt_ps,
                            wq_sb[:, k, cb * 128:(cb + 1) * 128],
                            qx_t[k],
                            start=(k == 0),
                            stop=(k == KT - 1),
                        )
                    nc.vector.tensor_copy(
                        g_qt[s][cb][:, ib2 * 512:(ib2 + 1) * 512], qt_ps
                    )

                def _kproj_mm(s, j0, bw, ct_t, cb):
                    kt_ps = ps.tile([128, 512], F32, tag="mm", bufs=2, name="kt_ps")
                    for k in range(KT):
                        nc.tensor.matmul(
                            kt_ps[:, 0:bw],
                            wk_sb[:, k, cb * 128:(cb + 1) * 128],
                            ct_t[k][:, 0:bw],
                            start=(k == 0),
                            stop=(k == KT - 1),
                        )
                    nc.vector.tensor_copy(g_kt[s][cb][:, j0:j0 + bw], kt_ps[:, 0:bw])

                def _vproj_mm(s, j0, ct_t, js):
                    # tag "mm", not "pv": when run as filler inside the
                    # attention stream both pv bufs are held by the open
                    # PV accumulators (tag-pv alloc would deadlock)
                    v_ps = ps.tile([128, CG], F32, tag="mm", bufs=2, name="v_ps")
                    for k in range(KT):
                        nc.tensor.matmul(
                            v_ps,
                            ct_t[k][:, js * 128:(js + 1) * 128],
                            wv_sb[:, k, :],
                            start=(k == 0),
                            stop=(k == KT - 1),
                        )
                    nc.vector.tensor_copy(
                        g_v[s][:, j0 // 128 + js].rearrange(
                            "p (h e) -> p h e", e=VSTR
                        )[:, :, 0:64],
                        v_ps.rearrange("p (h e) -> p h e", e=64),
                    )

                def _phase1_inline(s, qx0=None, ct_t=None):
                    j0, bw = jblocks[0]
                    if qx0 is None:
                        qx0 = _qt_dma(0)
                        ct_t = _ct_dma(j0, bw)
                    _qt_mm(s, 0, qx0, 0)
                    _qt_mm(s, 0, qx0, 1)
                    for cb in range(2):
                        _kproj_mm(s, j0, bw, ct_t, cb)
                    for js in range(bw // 128):
                        _vproj_mm(s, j0, ct_t, js)

                def _emit_st(s, hp, ib2, jt):
                    st = ps.tile([128, 1024], F32, tag="st", bufs=2, name="st")
                    for b in range(2):
                        nc.tensor.matmul(
                            st[:, b * 512:(b + 1) * 512],
                            g_kt[s][hp][b * 64:(b + 1) * 64, jt * 128:(jt + 1) * 128],
                            g_qt[s][hp][b * 64:(b + 1) * 64, ib2 * 512:(ib2 + 1) * 512],
                            start=True,
                            stop=True,
                        )
                    return st

                def _exp(st):
                    e = sb.tile([128, 1024], BF16, tag="et", bufs=4, name="e")
                    nc.scalar.activation(e, st, AF.Exp, scale=SCALE)
                    return e

                def _alloc_pvs():
                    return [
                        ps.tile([65, 512], F32, tag="pv", bufs=2, name="pv")
                        for _ in range(2)
                    ]

                def _pv_step(s, hp, jt, e, pvs):
                    for b in range(2):
                        h = 2 * hp + b
                        nc.tensor.matmul(
                            pvs[b],
                            g_v[s][:, jt, h * VSTR:h * VSTR + 65],
                            e[:, b * 512:(b + 1) * 512],
                            start=(jt == 0),
                            stop=(jt == JTC - 1),
                        )

                def _normalize(s, hp, ib2, pvs, tail=False):
                    """Denominator reciprocal runs in row 64 of a [65,512]
                    tile so input (pv PSUM row 64) and output share a base
                    partition (cross-partition reciprocal_approx_fast
                    mis-addresses on HW); the ot multiply then reads the pv
                    numerators straight from PSUM, so no pvc/dr copies."""
                    for b in range(2):
                        rec = sb.tile([65, 512], F32, tag="rec", bufs=2, name="rec")
                        nc.vector.reciprocal_approx_fast(
                            out=rec[64:65, :], in_=pvs[b][64:65, :])
                        den = sb.tile([64, 512], F32, tag="den", bufs=2, name="den")
                        nc.gpsimd.partition_broadcast(den, rec[64:65, :])
                        nc.vector.tensor_mul(
                            g_ot[s][hp][b * 64:(b + 1) * 64,
                                        ib2 * 512:(ib2 + 1) * 512],
                            pvs[b][0:64, :],
                            den,
                        )

                def _oproj_unit(s, ib2, it, m, copy_act=False, ot=None):
                    ot = ot if ot is not None else g_ot[s]
                    itg = ib2 * 4 + it
                    op = ps.tile([128, 512], F32, tag="mm", bufs=2, name="op")
                    for kk in range(2):
                        nc.tensor.matmul(
                            op,
                            ot[kk][:, itg * 128:(itg + 1) * 128],
                            wo_sb[:, kk, m * 512:(m + 1) * 512],
                            start=(kk == 0),
                            stop=(kk == 1),
                        )
                    osb = sb.tile([128, 512], BF16, tag="osb", bufs=3, name="osb")
                    if copy_act:
                        nc.scalar.activation(osb, op, AF.Copy)
                    else:
                        nc.vector.tensor_copy(osb, op)
                    nc.sync.dma_start(
                        out=outp[itg * 128:(itg + 1) * 128, m * 512:(m + 1) * 512],
                        in_=osb,
                    )

                def _oproj_fillers(s, ib2, units, jt0):
                    # each unit is 2 matmuls; jt0 leaves room for the
                    # producing normalize chain to finish
                    return [
                        (jt0 + i,
                         (lambda ib2=ib2, it=it, m=m: _oproj_unit(s, ib2, it, m)))
                        for i, (it, m) in enumerate(
                            ((u // 2, u % 2) for u in units)
                        )
                    ]

                def _qtproj_fillers(s, ib2):
                    # DMA issue up front; the 2x8 matmuls from jt=6
                    box = {}

                    def dma(ib2=ib2):
                        box["qx"] = _qt_dma(ib2)

                    return [(0, dma)] + [
                        (6 + 3 * cb,
                         (lambda ib2=ib2, cb=cb: _qt_mm(s, ib2, box["qx"], cb)))
                        for cb in range(2)
                    ]

                def _kv_fillers(s, blocks):
                    """K/V projection for jblocks[1:], interleaved into the
                    first attention block: block k (j-tiles 4k..4k+3) is
                    emitted by jt 4(k-1) so the ST prefetch (jt+2) never
                    outruns the kt/v writes."""
                    out = []
                    for k, (j0, bw) in blocks:
                        base = 4 * (k - 1)
                        box = {}

                        def dma(j0=j0, bw=bw, box=box):
                            box["ct"] = _ct_dma(j0, bw)

                        out.append((base, dma))
                        for cb in range(2):
                            out.append(
                                (base + cb, lambda j0=j0, bw=bw, cb=cb, box=box:
                                 _kproj_mm(s, j0, bw, box["ct"], cb))
                            )
                        for js in range(bw // 128):
                            out.append(
                                (base + 2 + js // 2, lambda j0=j0, js=js, box=box:
                                 _vproj_mm(s, j0, box["ct"], js))
                            )
                    return out

                def _phase1_fillers(s):
                    """Pass k's tail fillers computing pass k+1's (set s)
                    block-0 data. Returned as {block_idx: [(jt, fn), ...]}
                    for the last four attention blocks (idx 4..7)."""
                    j0, bw = jblocks[0]
                    box = {}

                    def qdma():
                        box["qx"] = _qt_dma(0)

                    def cdma(j0=j0, bw=bw):
                        box["ct"] = _ct_dma(j0, bw)

                    return {
                        3: [(9, qdma), (10, cdma)],
                        4: [(6, lambda: _qt_mm(s, 0, box["qx"], 0)),
                            (9, lambda: _qt_mm(s, 0, box["qx"], 1))],
                        5: [(6, lambda: _kproj_mm(s, j0, bw, box["ct"], 0)),
                            (9, lambda: _kproj_mm(s, j0, bw, box["ct"], 1))],
                        6: [(2 + 3 * js, lambda js=js: _vproj_mm(s, j0, box["ct"], js))
                            for js in range(bw // 128)],
                    }

                def _one_pass(s, defer_in=None, prep_next=None, tail_normalize=False):
                    """One full pass using activation-tile set s.

                    defer_in: the OTHER set's ot pair -- emit its final-block
                    oproj units here (their data has been ready since the
                    previous pass), and skip our own final-block oproj.
                    prep_next: set index to run phase-1 for, as tail fillers.
                    """
                    # per-block filler queues, keyed by block index in the
                    # global stream order [(hp0,ib0),(hp1,ib0),(hp0,ib1),...]
                    fq = {bi: [] for bi in range(2 * IB)}
                    fq[0] = _kv_fillers(s, list(enumerate(jblocks))[1:])
                    for ib2 in range(1, IB):
                        fq[2 * ib2] = _oproj_fillers(s, ib2 - 1, range(0, 6), 4)
                    for ib2 in range(0, IB - 1):
                        fl = list(_qtproj_fillers(s, ib2 + 1))
                        if ib2 >= 1:
                            fl += _oproj_fillers(s, ib2 - 1, range(6, 8), 2)
                        fl.sort(key=lambda x: x[0])
                        fq[2 * ib2 + 1] = fl
                    fq[2 * IB - 1] = _oproj_fillers(s, IB - 2, range(6, 8), 2)
                    if defer_in is not None:
                        dunits = [
                            (lambda u=u: _oproj_unit(
                                s, IB - 1, u // 2, u % 2, ot=defer_in))
                            for u in range(8)
                        ]
                        fq[2 * IB - 2] = sorted(
                            fq[2 * IB - 2] + list(enumerate(dunits[:4])),
                            key=lambda x: x[0])
                        fq[2 * IB - 1] = sorted(
                            fq[2 * IB - 1] + list(enumerate(dunits[4:])),
                            key=lambda x: x[0])
                    if prep_next is not None:
                        for bi, fl in _phase1_fillers(prep_next).items():
                            fq[bi] = sorted(fq[bi] + fl, key=lambda x: x[0])

                    # global attention stream: depth-2 st prefetch crossing
                    # block boundaries so exp never waits at a transition
                    steps = [(2 * ib2 + hp, hp, ib2, jt)
                             for ib2 in range(IB) for hp in range(2)
                             for jt in range(JTC)]
                    st_q = {}
                    for i in range(2):
                        _, hp, ib2, jt = steps[i]
                        st_q[i] = _emit_st(s, hp, ib2, jt)
                    pvs_by_block = {}
                    for idx, (bi, hp, ib2, jt) in enumerate(steps):
                        if jt == 0:
                            pvs_by_block[bi] = _alloc_pvs()
                        e = _exp(st_q.pop(idx))
                        if idx + 2 < len(steps):
                            _, hp2, ib22, jt2 = steps[idx + 2]
                            st_q[idx + 2] = _emit_st(s, hp2, ib22, jt2)
                        _pv_step(s, hp, jt, e, pvs_by_block[bi])
                        q = fq[bi]
                        while q and q[0][0] <= jt:
                            q.pop(0)[1]()
                        if jt == JTC - 1:
                            _normalize(s, hp, ib2, pvs_by_block.pop(bi),
                                       tail=(tail_normalize and bi == 2 * IB - 1))
                            for _, f in fq[bi]:
                                f()
                            fq[bi] = []

                    if defer_in is None:
                        for u in range(8):
                            _oproj_unit(s, IB - 1, u // 2, u % 2,
                                        copy_act=(u % 2 == 1))

                if reps == 1:
                    _phase1_inline(0, p1_qx, p1_ct)
                    _one_pass(0, tail_normalize=True)
                elif pipe:
                    # two unrolled passes per iteration with alternating
                    # buffer sets; each pass emits the OTHER set's final
                    # oproj (previous pass's data; garbage on iteration 0,
                    # harmless for timing-only loops) and the other set's
                    # phase-1 projections for the NEXT pass.
                    assert reps % 2 == 0, "pipelined builds need even reps"
                    _phase1_inline(0, p1_qx, p1_ct)
                    with tc.For_i(0, reps // 2, 1):
                        _one_pass(0, defer_in=g_ot[1], prep_next=1)
                        _one_pass(1, defer_in=g_ot[0], prep_next=0)
                else:
                    with tc.For_i(0, reps, 1):
                        _phase1_inline(0)
                        _one_pass(0, tail_normalize=True)

    nc.compile()
    return nc


def _nkc_for_mask(mask):
    """Compacted key count: max unmasked keys over batches, rounded to 128."""
    counts = [int((~mask[bi]).sum()) for bi in range(mask.shape[0])]
    nkc = max(max(counts), 1)
    nkc = min(((nkc + 127) // 128) * 128, NK)
    return nkc


def _bf16(a):
    import ml_dtypes

    return np.ascontiguousarray(a).astype(ml_dtypes.bfloat16)


def _prep_core_inputs(q, context, mask, Wq, Wkv, Wout, core, nkc=NK):
    bi, g = core // 4, core % 4
    c0 = g * CG
    JTC = nkc // 128
    keep_idx = np.nonzero(~mask[bi])[0]
    ctx_c = np.zeros((nkc, D), dtype=np.float32)
    ctx_c[: len(keep_idx)] = context[bi][keep_idx]
    # per-key keep flag in the V ones-column: padding keys have K=V=0 so
    # their exp(0)=1 weight contributes 0 to both numerator and denominator
    keep = np.zeros(nkc, dtype=np.float32)
    keep[: len(keep_idx)] = 1.0
    km = np.ascontiguousarray(keep.reshape(JTC, 128).T)       # [128, JTC]
    vo = np.zeros((128, JTC, HG, 4), dtype=np.float32)
    vo[:, :, :, 0] = km[:, :, None]
    return {
        "qT": _bf16(q[bi].T),
        "cT": _bf16(ctx_c.T),
        "wq": _bf16(Wq[:, c0:c0 + CG]),
        "wk": _bf16(Wkv[:, c0:c0 + CG]),
        "wv": _bf16(Wkv[:, D + c0:D + c0 + CG]),
        "wo": _bf16(Wout[c0:c0 + CG, :]),
        "vones": _bf16(vo.reshape(128, -1)),
    }


def kernel(q, context, mask, Wq, Wkv, Wout, b_out):
    from concourse.bass_utils import run_bass_kernel_spmd

    q = np.asarray(q, dtype=np.float32)
    context = np.asarray(context, dtype=np.float32)
    mask = np.asarray(mask)
    Wq = np.asarray(Wq, dtype=np.float32)
    Wkv = np.asarray(Wkv, dtype=np.float32)
    Wout = np.asarray(Wout, dtype=np.float32)
    b_out = np.asarray(b_out, dtype=np.float32)

    nkc = _nkc_for_mask(mask)
    key = ("nc", nkc)
    if key not in _CACHE:
        _CACHE[key] = build_nc(nkc=nkc)
    nc = _CACHE[key]
    _CACHE["nc"] = nc
    _CACHE["nkc"] = nkc

    in_maps = [
        _prep_core_inputs(q, context, mask, Wq, Wkv, Wout, c, nkc=nkc)
        for c in range(N_CORES)
    ]

    trace = bool(int(os.environ.get("BASS_ATTN_TRACE", "0")))
    res = run_bass_kernel_spmd(nc, in_maps, list(range(N_CORES)), trace=trace)
    _CACHE["last_results"] = res
    _CACHE["last_in_maps"] = in_maps

    out = np.empty((B, NQ, D), dtype=np.float32)
    for bi in range(B):
        acc = res.results[4 * bi]["outp"].astype(np.float32).copy()
        for g in range(1, 4):
            acc += res.results[4 * bi + g]["outp"].astype(np.float32)
        out[bi] = acc + b_out[None, :]
    return out


# revision 21
# speedup vs baseline: 1.5512x; 1.2127x over previous
"""Trainium2 Bass kernel for nn_Attention_59287728554369.

Multi-head cross-attention, b=2, nq=nk=2048, 16 heads x 64 dim, d_model=1024.
Sharding: batch (2) x head-groups (4 heads each) -> 8 cores.
Each core computes q/k/v projections for its 4 heads, fused masked softmax
attention, and a partial output projection; host sums the 4 partials per batch.

v4 (from v2's 162us HW / 178us sim; both PE (~150us incl ldweights) and
ACT (~127us exp floor) are near-saturated, so gains here are structural):
- mask/padding moved entirely into the V ones-column (vones=keep-flag per
  key): padding keys have zero context rows -> K=V=0 -> exp(0)=1 contributes
  1*0 to numerator and 0 to denominator. Removes the kb bias input and the
  per-jt exp bias coupling. Exact (verified identical rel err).
- attention restructured from per-block loops into ONE global (block, jt)
  stream with the depth-2 st prefetch crossing block boundaries, so the ACT
  exp stream never breaks at hp/iblock transitions.
- pipe mode: pass k emits pass k+1's phase-1 work (q/ct block-0 DMAs +
  qt/kt/v block-0 projections, double-buffered per-set activation tiles) as
  PE filler inside its last attention blocks, in addition to deferring its
  own final oproj to pass k+1 (v2 mechanism). ACT then flows between passes
  with no projection preamble bubble.
- normalize: dr/reciprocal/broadcast as v2 (reciprocal_approx_fast needs
  SBUF operands + base partition 0 — PSUM input NaNs on HW), but the final
  ot multiply reads the pv numerators straight from PSUM (pvc copy dropped,
  ~10us/pass less DVE).
- startup: phase-1 qT/cT block-0 DMAs split per k-tile and queued right
  after wq so the first projection matmul starts ~2us in (single-shot
  reps=1 executions; loop steady-state unaffected).
- vones DMA hoisted out of the pass loop (written once per buffer set).
- bf16 everywhere, compacted keys, merged 3D DMAs, denominator via
  ones-augmented V column (v2).
Known dead ends (measured): fp8 PV/ST (DoubleRow) — error scales to ~2-6%
vs the 2e-2 gate; exp pair-batching — PSUM can't double-buffer [128,2048]
pairs, WAR stall exceeds the 352-cyc/instr ACT overhead saved; 64-row st
tile pairs overlap only ~15% on HW (microbench), not the 2x the tiling doc
suggests.
"""
import os
import sys

sys.path.insert(0, "/opt/trn_rl_repo")

import numpy as np

import concourse.bass as bass  # noqa: F401
import concourse.tile as tile
from concourse import bacc, mybir

F32 = mybir.dt.float32
BF16 = mybir.dt.bfloat16
AF = mybir.ActivationFunctionType

# Problem constants (hardcoded per contest rules)
B = 2
NQ = 2048
NK = 2048
D = 1024          # d_model
H = 16            # total heads
DH = 64           # head dim
HG = 4            # heads per core
CG = HG * DH      # channels per core = 256
N_CORES = 8
SCALE = DH ** -0.5
# V per-head stride: 64 data + 1 ones + 3 pad so each head slice starts
# 8-byte aligned in bf16 (65*2=130B bases mis-address PE ldweights on HW)
VSTR = 68

_CACHE = {}


def build_nc(reps=1, nkc=NK):
    """Build the single-core Bass program (identical across cores).

    nkc: compacted key count (multiple of 128, <= NK).
    reps>1 wraps the computation in an on-device For_i loop (same buffers) so
    test harnesses can measure marginal wall time per rep = HW exec time.
    """
    assert nkc % 128 == 0 and 128 <= nkc <= NK
    JTC = nkc // 128               # 128-wide j tiles
    # j blocks for the projections: full 512s plus one remainder block
    jblocks = [(s, 512) for s in range(0, nkc - nkc % 512, 512)]
    if nkc % 512:
        jblocks.append((nkc - nkc % 512, nkc % 512))

    nc = bacc.Bacc("TRN2", target_bir_lowering=False, debug=False)

    qT = nc.dram_tensor("qT", [D, NQ], BF16, kind="ExternalInput").ap()
    cT = nc.dram_tensor("cT", [D, nkc], BF16, kind="ExternalInput").ap()
    wq = nc.dram_tensor("wq", [D, CG], BF16, kind="ExternalInput").ap()
    wk = nc.dram_tensor("wk", [D, CG], BF16, kind="ExternalInput").ap()
    wv = nc.dram_tensor("wv", [D, CG], BF16, kind="ExternalInput").ap()
    wo = nc.dram_tensor("wo", [CG, D], BF16, kind="ExternalInput").ap()
    vones = nc.dram_tensor("vones", [128, JTC * HG * 4], BF16, kind="ExternalInput").ap()
    outp = nc.dram_tensor("outp", [NQ, D], BF16, kind="ExternalOutput").ap()

    KT = 8   # k tiles over d_model
    IB = 4   # 512-wide i blocks

    with tile.TileContext(nc) as tc:
        with tc.tile_pool(name="sb", bufs=1) as sb:
            # ---- persistent SBUF tensors; DMA issue order puts the first
            # projection's dependencies (wq, wk, qT block 0, cT block 0)
            # at the head of the queue ----
            wqr = wq.rearrange("(t p) c -> p t c", p=128)
            wq_sb = sb.tile([128, KT, CG], BF16, bufs=1)
            nc.sync.dma_start(out=wq_sb[:, :, 0:128], in_=wqr[:, :, 0:128])

            def _qt_dma(ib2):
                a = sb.tile([128, KT, 512], BF16, tag="act", bufs=5, name="act")
                nc.sync.dma_start(
                    out=a,
                    in_=qT.rearrange("(t p) i -> p t i", p=128)[
                        :, :, ib2 * 512:(ib2 + 1) * 512],
                )
                return [a[:, k, :] for k in range(KT)]

            def _ct_dma(j0, bw):
                a = sb.tile([128, KT, 512], BF16, tag="act", bufs=5, name="act")
                nc.sync.dma_start(
                    out=a[:, :, 0:bw],
                    in_=cT.rearrange("(t p) j -> p t j", p=128)[:, :, j0:j0 + bw],
                )
                return [a[:, k, :] for k in range(KT)]

            # phase-1 block-0 inputs queued before the remaining weights,
            # chunked per k-tile so the first projection matmul can start
            # after ~0.6MB instead of after the full phase-1 payload
            # (matters for single-shot reps=1 executions)
            qTr = qT.rearrange("(t p) i -> p t i", p=128)
            a1 = sb.tile([128, KT, 512], BF16, tag="act", bufs=5, name="act")
            for k in range(KT):
                nc.sync.dma_start(out=a1[:, k, :], in_=qTr[:, k, 0:512])
            p1_qx = [a1[:, k, :] for k in range(KT)]

            wkr = wk.rearrange("(t p) c -> p t c", p=128)
            wk_sb = sb.tile([128, KT, CG], BF16, bufs=1)
            nc.sync.dma_start(out=wk_sb[:, :, 0:128], in_=wkr[:, :, 0:128])
            cTr = cT.rearrange("(t p) j -> p t j", p=128)
            bw0 = jblocks[0][1]
            a2 = sb.tile([128, KT, 512], BF16, tag="act", bufs=5, name="act")
            for k in range(KT):
                nc.sync.dma_start(out=a2[:, k, 0:bw0], in_=cTr[:, k, 0:bw0])
            p1_ct = [a2[:, k, :] for k in range(KT)]

            # second halves of the q/k projection weights
            nc.sync.dma_start(out=wq_sb[:, :, 128:256], in_=wqr[:, :, 128:256])
            nc.sync.dma_start(out=wk_sb[:, :, 128:256], in_=wkr[:, :, 128:256])

            wv_sb = sb.tile([128, KT, CG], BF16, bufs=1)
            nc.sync.dma_start(out=wv_sb, in_=wv.rearrange("(t p) c -> p t c", p=128))
            wo_sb = sb.tile([128, 2, D], BF16, bufs=1)
            nc.sync.dma_start(out=wo_sb, in_=wo.rearrange("(t p) m -> p t m", p=128))

            pipe = bool(int(os.environ.get("BASS_ATTN_PIPE", "1"))) and reps > 1
            NSETS = 2 if pipe else 1
            # per-set persistent activation tensors (pipe: double-buffered so
            # pass k's tail can project pass k+1's block-0 data)
            g_kt = [[sb.tile([128, nkc], BF16, bufs=1, name=f"kt{s}{i}")
                     for i in range(2)] for s in range(NSETS)]
            g_qt = [[sb.tile([128, NQ], BF16, bufs=1, name=f"qt{s}{i}")
                     for i in range(2)] for s in range(NSETS)]
            g_v = [sb.tile([128, JTC, HG * VSTR], BF16, bufs=1, name=f"v{s}")
                   for s in range(NSETS)]
            g_ot = [[sb.tile([128, NQ], BF16, bufs=1, name=f"ot{s}{i}")
                     for i in range(2)] for s in range(NSETS)]
            for s in range(NSETS):
                nc.sync.dma_start(
                    out=g_v[s].rearrange("p t (h e) -> p t h e", e=VSTR)[:, :, :, 64:VSTR],
                    in_=vones.rearrange("p (t h e) -> p t h e", h=HG, e=4),
                )

            with tc.tile_pool(name="ps", bufs=1, space="PSUM") as ps:

                def _qt_mm(s, ib2, qx_t, cb):
                    qt_ps = ps.tile([128, 512], F32, tag="mm", bufs=2, name="qt_ps")
                    for k in range(KT):
                        nc.tensor.matmul(
                            qt_ps,
                            wq_sb[:, k, cb * 128:(cb + 1) * 128],
                            qx_t[k],
                            start=(k == 0),
                            stop=(k == KT - 1),
                        )
                    nc.vector.tensor_copy(
                        g_qt[s][cb][:, ib2 * 512:(ib2 + 1) * 512], qt_ps
                    )

                def _kproj_mm(s, j0, bw, ct_t, cb):
                    kt_ps = ps.tile([128, 512], F32, tag="mm", bufs=2, name="kt_ps")
                    for k in range(KT):
                        nc.tensor.matmul(
                            kt_ps[:, 0:bw],
                            wk_sb[:, k, cb * 128:(cb + 1) * 128],
                            ct_t[k][:, 0:bw],
                            start=(k == 0),
                            stop=(k == KT - 1),
                        )
                    nc.vector.tensor_copy(g_kt[s][cb][:, j0:j0 + bw], kt_ps[:, 0:bw])

                def _vproj_mm(s, j0, ct_t, js):
                    # tag "mm", not "pv": when run as filler inside the
                    # attention stream both pv bufs are held by the open
                    # PV accumulators (tag-pv alloc would deadlock)
                    v_ps = ps.tile([128, CG], F32, tag="mm", bufs=2, name="v_ps")
                    for k in range(KT):
                        nc.tensor.matmul(
                            v_ps,
                            ct_t[k][:, js * 128:(js + 1) * 128],
                            wv_sb[:, k, :],
                            start=(k == 0),
                            stop=(k == KT - 1),
                        )
                    nc.vector.tensor_copy(
                        g_v[s][:, j0 // 128 + js].rearrange(
                            "p (h e) -> p h e", e=VSTR
                        )[:, :, 0:64],
                        v_ps.rearrange("p (h e) -> p h e", e=64),
                    )

                def _phase1_inline(s, qx0=None, ct_t=None, lean=False):
                    """lean=True emits only what the stream head needs (qt/kt
                    cb0 + v js0/js1) and returns the rest as block-0 fillers,
                    shortening the single-shot startup critical path."""
                    j0, bw = jblocks[0]
                    if qx0 is None:
                        qx0 = _qt_dma(0)
                        ct_t = _ct_dma(j0, bw)
                    if lean:
                        _qt_mm(s, 0, qx0, 0)
                        _kproj_mm(s, j0, bw, ct_t, 0)
                        _vproj_mm(s, j0, ct_t, 0)
                        _vproj_mm(s, j0, ct_t, 1)
                        # cb1 data is first needed when block 1's sts are
                        # prefetched (stream idx 11), so emit late enough
                        # not to starve block 0's own st/pv stream
                        return [
                            (1, lambda: _vproj_mm(s, j0, ct_t, 2)),
                            (2, lambda: _vproj_mm(s, j0, ct_t, 3)),
                            (5, lambda: _qt_mm(s, 0, qx0, 1)),
                            (7, lambda: _kproj_mm(s, j0, bw, ct_t, 1)),
                        ]
                    _qt_mm(s, 0, qx0, 0)
                    _qt_mm(s, 0, qx0, 1)
                    for cb in range(2):
                        _kproj_mm(s, j0, bw, ct_t, cb)
                    for js in range(bw // 128):
                        _vproj_mm(s, j0, ct_t, js)
                    return []

                def _emit_st(s, hp, ib2, jt):
                    st = ps.tile([128, 1024], F32, tag="st", bufs=2, name="st")
                    for b in range(2):
                        nc.tensor.matmul(
                            st[:, b * 512:(b + 1) * 512],
                            g_kt[s][hp][b * 64:(b + 1) * 64, jt * 128:(jt + 1) * 128],
                            g_qt[s][hp][b * 64:(b + 1) * 64, ib2 * 512:(ib2 + 1) * 512],
                            start=True,
                            stop=True,
                        )
                    return st

                def _exp(st):
                    e = sb.tile([128, 1024], BF16, tag="et", bufs=4, name="e")
                    nc.scalar.activation(e, st, AF.Exp, scale=SCALE)
                    return e

                def _alloc_pvs():
                    return [
                        ps.tile([65, 512], F32, tag="pv", bufs=2, name="pv")
                        for _ in range(2)
                    ]

                def _pv_step(s, hp, jt, e, pvs):
                    for b in range(2):
                        h = 2 * hp + b
                        nc.tensor.matmul(
                            pvs[b],
                            g_v[s][:, jt, h * VSTR:h * VSTR + 65],
                            e[:, b * 512:(b + 1) * 512],
                            start=(jt == 0),
                            stop=(jt == JTC - 1),
                        )

                def _normalize(s, hp, ib2, pvs, tail=False):
                    """Denominator row copied to partition 0 first
                    (reciprocal_approx_fast mis-addresses cross-partition on
                    HW, and needs SBUF operands); the ot multiply reads the
                    pv numerators straight from PSUM, skipping the old
                    [65,512] pvc copy."""
                    for b in range(2):
                        dr = sb.tile([1, 512], F32, tag="dr", bufs=2, name="dr")
                        nc.vector.tensor_copy(dr, pvs[b][64:65, :])
                        rec = sb.tile([1, 512], F32, tag="rec", bufs=2, name="rec")
                        nc.vector.reciprocal_approx_fast(out=rec, in_=dr)
                        den = sb.tile([64, 512], F32, tag="den", bufs=2, name="den")
                        nc.gpsimd.partition_broadcast(den, rec[0:1, :])
                        nc.vector.tensor_mul(
                            g_ot[s][hp][b * 64:(b + 1) * 64,
                                        ib2 * 512:(ib2 + 1) * 512],
                            pvs[b][0:64, :],
                            den,
                        )

                def _oproj_unit(s, ib2, it, m, copy_act=False, ot=None):
                    ot = ot if ot is not None else g_ot[s]
                    itg = ib2 * 4 + it
                    op = ps.tile([128, 512], F32, tag="mm", bufs=2, name="op")
                    for kk in range(2):
                        nc.tensor.matmul(
                            op,
                            ot[kk][:, itg * 128:(itg + 1) * 128],
                            wo_sb[:, kk, m * 512:(m + 1) * 512],
                            start=(kk == 0),
                            stop=(kk == 1),
                        )
                    osb = sb.tile([128, 512], BF16, tag="osb", bufs=3, name="osb")
                    if copy_act:
                        nc.scalar.activation(osb, op, AF.Copy)
                    else:
                        nc.vector.tensor_copy(osb, op)
                    nc.sync.dma_start(
                        out=outp[itg * 128:(itg + 1) * 128, m * 512:(m + 1) * 512],
                        in_=osb,
                    )

                def _oproj_fillers(s, ib2, units, jt0):
                    # each unit is 2 matmuls; jt0 leaves room for the
                    # producing normalize chain to finish
                    return [
                        (jt0 + i,
                         (lambda ib2=ib2, it=it, m=m: _oproj_unit(s, ib2, it, m)))
                        for i, (it, m) in enumerate(
                            ((u // 2, u % 2) for u in units)
                        )
                    ]

                def _qtproj_fillers(s, ib2):
                    # DMA issue up front; the 2x8 matmuls from jt=6
                    box = {}

                    def dma(ib2=ib2):
                        box["qx"] = _qt_dma(ib2)

                    return [(0, dma)] + [
                        (6 + 3 * cb,
                         (lambda ib2=ib2, cb=cb: _qt_mm(s, ib2, box["qx"], cb)))
                        for cb in range(2)
                    ]

                def _kv_fillers(s, blocks):
                    """K/V projection for jblocks[1:], interleaved into the
                    first attention block: block k (j-tiles 4k..4k+3) is
                    emitted by jt 4(k-1) so the ST prefetch (jt+2) never
                    outruns the kt/v writes."""
                    out = []
                    for k, (j0, bw) in blocks:
                        base = 4 * (k - 1)
                        box = {}

                        def dma(j0=j0, bw=bw, box=box):
                            box["ct"] = _ct_dma(j0, bw)

                        out.append((base, dma))
                        for cb in range(2):
                            out.append(
                                (base + cb, lambda j0=j0, bw=bw, cb=cb, box=box:
                                 _kproj_mm(s, j0, bw, box["ct"], cb))
                            )
                        for js in range(bw // 128):
                            out.append(
                                (base + 2 + js // 2, lambda j0=j0, js=js, box=box:
                                 _vproj_mm(s, j0, box["ct"], js))
                            )
                    return out

                def _phase1_fillers(s):
                    """Pass k's tail fillers computing pass k+1's (set s)
                    block-0 data. Returned as {block_idx: [(jt, fn), ...]}
                    for the last four attention blocks (idx 4..7)."""
                    j0, bw = jblocks[0]
                    box = {}

                    def qdma():
                        box["qx"] = _qt_dma(0)

                    def cdma(j0=j0, bw=bw):
                        box["ct"] = _ct_dma(j0, bw)

                    return {
                        3: [(9, qdma), (10, cdma)],
                        4: [(6, lambda: _qt_mm(s, 0, box["qx"], 0)),
                            (9, lambda: _qt_mm(s, 0, box["qx"], 1))],
                        5: [(6, lambda: _kproj_mm(s, j0, bw, box["ct"], 0)),
                            (9, lambda: _kproj_mm(s, j0, bw, box["ct"], 1))],
                        6: [(2 + 3 * js, lambda js=js: _vproj_mm(s, j0, box["ct"], js))
                            for js in range(bw // 128)],
                    }

                def _one_pass(s, defer_in=None, prep_next=None,
                              tail_normalize=False, extra_f0=()):
                    """One full pass using activation-tile set s.

                    defer_in: the OTHER set's ot pair -- emit its final-block
                    oproj units here (their data has been ready since the
                    previous pass), and skip our own final-block oproj.
                    prep_next: set index to run phase-1 for, as tail fillers.
                    """
                    # per-block filler queues, keyed by block index in the
                    # global stream order [(hp0,ib0),(hp1,ib0),(hp0,ib1),...]
                    fq = {bi: [] for bi in range(2 * IB)}
                    fq[0] = sorted(
                        list(extra_f0) + _kv_fillers(s, list(enumerate(jblocks))[1:]),
                        key=lambda x: x[0])
                    for ib2 in range(1, IB):
                        fq[2 * ib2] = _oproj_fillers(s, ib2 - 1, range(0, 6), 4)
                    for ib2 in range(0, IB - 1):
                        fl = list(_qtproj_fillers(s, ib2 + 1))
                        if ib2 >= 1:
                            fl += _oproj_fillers(s, ib2 - 1, range(6, 8), 2)
                        fl.sort(key=lambda x: x[0])
                        fq[2 * ib2 + 1] = fl
                    fq[2 * IB - 1] = _oproj_fillers(s, IB - 2, range(6, 8), 2)
                    if defer_in is not None:
                        dunits = [
                            (lambda u=u: _oproj_unit(
                                s, IB - 1, u // 2, u % 2, ot=defer_in))
                            for u in range(8)
                        ]
                        fq[2 * IB - 2] = sorted(
                            fq[2 * IB - 2] + list(enumerate(dunits[:4])),
                            key=lambda x: x[0])
                        fq[2 * IB - 1] = sorted(
                            fq[2 * IB - 1] + list(enumerate(dunits[4:])),
                            key=lambda x: x[0])
                    if prep_next is not None:
                        for bi, fl in _phase1_fillers(prep_next).items():
                            fq[bi] = sorted(fq[bi] + fl, key=lambda x: x[0])

                    # global attention stream: depth-2 st prefetch crossing
                    # block boundaries so exp never waits at a transition
                    steps = [(2 * ib2 + hp, hp, ib2, jt)
                             for ib2 in range(IB) for hp in range(2)
                             for jt in range(JTC)]
                    st_q = {}
                    for i in range(2):
                        _, hp, ib2, jt = steps[i]
                        st_q[i] = _emit_st(s, hp, ib2, jt)
                    pvs_by_block = {}
                    for idx, (bi, hp, ib2, jt) in enumerate(steps):
                        if jt == 0:
                            pvs_by_block[bi] = _alloc_pvs()
                        e = _exp(st_q.pop(idx))
                        if idx + 2 < len(steps):
                            _, hp2, ib22, jt2 = steps[idx + 2]
                            st_q[idx + 2] = _emit_st(s, hp2, ib22, jt2)
                        _pv_step(s, hp, jt, e, pvs_by_block[bi])
                        q = fq[bi]
                        while q and q[0][0] <= jt:
                            q.pop(0)[1]()
                        if jt == JTC - 1:
                            _normalize(s, hp, ib2, pvs_by_block.pop(bi),
                                       tail=(tail_normalize and bi == 2 * IB - 1))
                            for _, f in fq[bi]:
                                f()
                            fq[bi] = []

                    if defer_in is None:
                        for u in range(8):
                            _oproj_unit(s, IB - 1, u // 2, u % 2,
                                        copy_act=(u % 2 == 1))

                if reps == 1:
                    f0 = _phase1_inline(0, p1_qx, p1_ct, lean=True)
                    _one_pass(0, tail_normalize=True, extra_f0=f0)
                elif pipe:
                    # two unrolled passes per iteration with alternating
                    # buffer sets; each pass emits the OTHER set's final
                    # oproj (previous pass's data; garbage on iteration 0,
                    # harmless for timing-only loops) and the other set's
                    # phase-1 projections for the NEXT pass.
                    assert reps % 2 == 0, "pipelined builds need even reps"
                    _phase1_inline(0, p1_qx, p1_ct)
                    with tc.For_i(0, reps // 2, 1):
                        _one_pass(0, defer_in=g_ot[1], prep_next=1)
                        _one_pass(1, defer_in=g_ot[0], prep_next=0)
                else:
                    with tc.For_i(0, reps, 1):
                        _phase1_inline(0)
                        _one_pass(0, tail_normalize=True)

    nc.compile()
    return nc


def _nkc_for_mask(mask):
    """Compacted key count: max unmasked keys over batches, rounded to 128."""
    counts = [int((~mask[bi]).sum()) for bi in range(mask.shape[0])]
    nkc = max(max(counts), 1)
    nkc = min(((nkc + 127) // 128) * 128, NK)
    return nkc


def _bf16(a):
    import ml_dtypes

    return np.ascontiguousarray(a).astype(ml_dtypes.bfloat16)


def _prep_core_inputs(q, context, mask, Wq, Wkv, Wout, core, nkc=NK):
    bi, g = core // 4, core % 4
    c0 = g * CG
    JTC = nkc // 128
    keep_idx = np.nonzero(~mask[bi])[0]
    ctx_c = np.zeros((nkc, D), dtype=np.float32)
    ctx_c[: len(keep_idx)] = context[bi][keep_idx]
    # per-key keep flag in the V ones-column: padding keys have K=V=0 so
    # their exp(0)=1 weight contributes 0 to both numerator and denominator
    keep = np.zeros(nkc, dtype=np.float32)
    keep[: len(keep_idx)] = 1.0
    km = np.ascontiguousarray(keep.reshape(JTC, 128).T)       # [128, JTC]
    vo = np.zeros((128, JTC, HG, 4), dtype=np.float32)
    vo[:, :, :, 0] = km[:, :, None]
    return {
        "qT": _bf16(q[bi].T),
        "cT": _bf16(ctx_c.T),
        "wq": _bf16(Wq[:, c0:c0 + CG]),
        "wk": _bf16(Wkv[:, c0:c0 + CG]),
        "wv": _bf16(Wkv[:, D + c0:D + c0 + CG]),
        "wo": _bf16(Wout[c0:c0 + CG, :]),
        "vones": _bf16(vo.reshape(128, -1)),
    }


def kernel(q, context, mask, Wq, Wkv, Wout, b_out):
    from concourse.bass_utils import run_bass_kernel_spmd

    q = np.asarray(q, dtype=np.float32)
    context = np.asarray(context, dtype=np.float32)
    mask = np.asarray(mask)
    Wq = np.asarray(Wq, dtype=np.float32)
    Wkv = np.asarray(Wkv, dtype=np.float32)
    Wout = np.asarray(Wout, dtype=np.float32)
    b_out = np.asarray(b_out, dtype=np.float32)

    nkc = _nkc_for_mask(mask)
    key = ("nc", nkc)
    if key not in _CACHE:
        _CACHE[key] = build_nc(nkc=nkc)
    nc = _CACHE[key]
    _CACHE["nc"] = nc
    _CACHE["nkc"] = nkc

    in_maps = [
        _prep_core_inputs(q, context, mask, Wq, Wkv, Wout, c, nkc=nkc)
        for c in range(N_CORES)
    ]

    trace = bool(int(os.environ.get("BASS_ATTN_TRACE", "0")))
    res = run_bass_kernel_spmd(nc, in_maps, list(range(N_CORES)), trace=trace)
    _CACHE["last_results"] = res
    _CACHE["last_in_maps"] = in_maps

    out = np.empty((B, NQ, D), dtype=np.float32)
    for bi in range(B):
        acc = res.results[4 * bi]["outp"].astype(np.float32).copy()
        for g in range(1, 4):
            acc += res.results[4 * bi + g]["outp"].astype(np.float32)
        out[bi] = acc + b_out[None, :]
    return out


# revision 25
# speedup vs baseline: 2.3084x; 1.4881x over previous
"""Trainium2 Bass kernel for nn_Attention_59287728554369.

Multi-head cross-attention, b=2, nq=nk=2048, 16 heads x 64 dim, d_model=1024.
Sharding: batch (2) x head-groups (4 heads each) -> 8 cores.
Each core computes q/k/v projections for its 4 heads, fused masked softmax
attention, and a partial output projection; host sums the 4 partials per batch.

v4 (from v2's 162us HW / 178us sim; both PE (~150us incl ldweights) and
ACT (~127us exp floor) are near-saturated, so gains here are structural):
- mask/padding moved entirely into the V ones-column (vones=keep-flag per
  key): padding keys have zero context rows -> K=V=0 -> exp(0)=1 contributes
  1*0 to numerator and 0 to denominator. Removes the kb bias input and the
  per-jt exp bias coupling. Exact (verified identical rel err).
- attention restructured from per-block loops into ONE global (block, jt)
  stream with the depth-2 st prefetch crossing block boundaries, so the ACT
  exp stream never breaks at hp/iblock transitions.
- pipe mode: pass k emits pass k+1's phase-1 work (q/ct block-0 DMAs +
  qt/kt/v block-0 projections, double-buffered per-set activation tiles) as
  PE filler inside its last attention blocks, in addition to deferring its
  own final oproj to pass k+1 (v2 mechanism). ACT then flows between passes
  with no projection preamble bubble.
- normalize kept in v2 form: pvc copy first (frees the pv PSUM bank for the
  next block's accumulation; reading PSUM from the final multiply instead
  holds the bank ~2us longer and cost ~9us/pass in flat-pstate sim), dr row
  copied to partition 0 (reciprocal_approx_fast needs SBUF operands with
  equal in/out base partitions — PSUM input NaNs on HW).
- startup: wq/wk DMAs split per cb half and phase-1 qT/cT block-0 DMAs
  split per k-tile, queued in first-use order so the first projection
  matmul starts ~1-2us in (single-shot reps=1 executions; loop steady-state
  unaffected). BASS_ATTN_LEAN=1 additionally defers phase-1 cb1/vproj work
  into block-0 fillers — measured ~3us WORSE in flat-pstate sim (filler
  congestion starves the st stream), so it defaults off.
- vones DMA hoisted out of the pass loop (written once per buffer set).
- bf16 everywhere, compacted keys, merged 3D DMAs, denominator via
  ones-augmented V column (v2).
Known dead ends (measured): fp8 PV/ST (DoubleRow) — error scales to ~2-6%
vs the 2e-2 gate; exp pair-batching — PSUM can't double-buffer [128,2048]
pairs, WAR stall exceeds the 352-cyc/instr ACT overhead saved; 64-row st
tile pairs overlap only ~15% on HW (microbench), not the 2x the tiling doc
suggests.
"""
import os
import sys

sys.path.insert(0, "/opt/trn_rl_repo")

import numpy as np

import concourse.bass as bass  # noqa: F401
import concourse.tile as tile
from concourse import bacc, mybir

F32 = mybir.dt.float32
BF16 = mybir.dt.bfloat16
AF = mybir.ActivationFunctionType

# Problem constants (hardcoded per contest rules)
B = 2
NQ = 2048
NK = 2048
D = 1024          # d_model
H = 16            # total heads
DH = 64           # head dim
HG = 4            # heads per core
CG = HG * DH      # channels per core = 256
N_CORES = 8
SCALE = DH ** -0.5
# V per-head stride: 64 data + 1 ones + 3 pad so each head slice starts
# 8-byte aligned in bf16 (65*2=130B bases mis-address PE ldweights on HW)
VSTR = 68

_CACHE = {}


def build_nc(reps=1, nkc=NK):
    """Build the single-core Bass program (identical across cores).

    nkc: compacted key count (multiple of 128, <= NK).
    reps>1 wraps the computation in an on-device For_i loop (same buffers) so
    test harnesses can measure marginal wall time per rep = HW exec time.
    """
    assert nkc % 128 == 0 and 128 <= nkc <= NK
    JTC = nkc // 128               # 128-wide j tiles
    # j blocks for the projections: full 512s plus one remainder block
    jblocks = [(s, 512) for s in range(0, nkc - nkc % 512, 512)]
    if nkc % 512:
        jblocks.append((nkc - nkc % 512, nkc % 512))

    nc = bacc.Bacc("TRN2", target_bir_lowering=False, debug=False)

    qT = nc.dram_tensor("qT", [D, NQ], BF16, kind="ExternalInput").ap()
    cT = nc.dram_tensor("cT", [D, nkc], BF16, kind="ExternalInput").ap()
    wq = nc.dram_tensor("wq", [D, CG], BF16, kind="ExternalInput").ap()
    wk = nc.dram_tensor("wk", [D, CG], BF16, kind="ExternalInput").ap()
    wv = nc.dram_tensor("wv", [D, CG], BF16, kind="ExternalInput").ap()
    wo = nc.dram_tensor("wo", [CG, D], BF16, kind="ExternalInput").ap()
    vones = nc.dram_tensor("vones", [128, JTC * HG * 4], BF16, kind="ExternalInput").ap()
    outp = nc.dram_tensor("outp", [NQ, D], BF16, kind="ExternalOutput").ap()

    KT = 8   # k tiles over d_model
    IB = 4   # 512-wide i blocks

    with tile.TileContext(nc) as tc:
        with tc.tile_pool(name="sb", bufs=1) as sb:
            # ---- persistent SBUF tensors; DMA issue order puts the first
            # projection's dependencies (wq, wk, qT block 0, cT block 0)
            # at the head of the queue ----
            wqr = wq.rearrange("(t p) c -> p t c", p=128)
            wq_sb = sb.tile([128, KT, CG], BF16, bufs=1)
            nc.sync.dma_start(out=wq_sb[:, :, 0:128], in_=wqr[:, :, 0:128])

            def _qt_dma(ib2):
                a = sb.tile([128, KT, 512], BF16, tag="act", bufs=5, name="act")
                nc.sync.dma_start(
                    out=a,
                    in_=qT.rearrange("(t p) i -> p t i", p=128)[
                        :, :, ib2 * 512:(ib2 + 1) * 512],
                )
                return [a[:, k, :] for k in range(KT)]

            def _ct_dma(j0, bw):
                a = sb.tile([128, KT, 512], BF16, tag="act", bufs=5, name="act")
                nc.sync.dma_start(
                    out=a[:, :, 0:bw],
                    in_=cT.rearrange("(t p) j -> p t j", p=128)[:, :, j0:j0 + bw],
                )
                return [a[:, k, :] for k in range(KT)]

            # phase-1 block-0 inputs queued before the remaining weights,
            # chunked per k-tile so the first projection matmul can start
            # after ~0.6MB instead of after the full phase-1 payload
            # (matters for single-shot reps=1 executions)
            qTr = qT.rearrange("(t p) i -> p t i", p=128)
            a1 = sb.tile([128, KT, 512], BF16, tag="act", bufs=5, name="act")
            for k in range(KT):
                nc.sync.dma_start(out=a1[:, k, :], in_=qTr[:, k, 0:512])
            p1_qx = [a1[:, k, :] for k in range(KT)]

            wkr = wk.rearrange("(t p) c -> p t c", p=128)
            wk_sb = sb.tile([128, KT, CG], BF16, bufs=1)
            nc.sync.dma_start(out=wk_sb[:, :, 0:128], in_=wkr[:, :, 0:128])
            cTr = cT.rearrange("(t p) j -> p t j", p=128)
            bw0 = jblocks[0][1]
            a2 = sb.tile([128, KT, 512], BF16, tag="act", bufs=5, name="act")
            for k in range(KT):
                nc.sync.dma_start(out=a2[:, k, 0:bw0], in_=cTr[:, k, 0:bw0])
            p1_ct = [a2[:, k, :] for k in range(KT)]

            # second halves of the q/k projection weights
            nc.sync.dma_start(out=wq_sb[:, :, 128:256], in_=wqr[:, :, 128:256])
            nc.sync.dma_start(out=wk_sb[:, :, 128:256], in_=wkr[:, :, 128:256])

            wv_sb = sb.tile([128, KT, CG], BF16, bufs=1)
            nc.sync.dma_start(out=wv_sb, in_=wv.rearrange("(t p) c -> p t c", p=128))
            wo_sb = sb.tile([128, 2, D], BF16, bufs=1)
            nc.sync.dma_start(out=wo_sb, in_=wo.rearrange("(t p) m -> p t m", p=128))

            pipe = bool(int(os.environ.get("BASS_ATTN_PIPE", "1"))) and reps > 1
            NSETS = 2 if pipe else 1
            # per-set persistent activation tensors (pipe: double-buffered so
            # pass k's tail can project pass k+1's block-0 data)
            g_kt = [[sb.tile([128, nkc], BF16, bufs=1, name=f"kt{s}{i}")
                     for i in range(2)] for s in range(NSETS)]
            g_qt = [[sb.tile([128, NQ], BF16, bufs=1, name=f"qt{s}{i}")
                     for i in range(2)] for s in range(NSETS)]
            g_v = [sb.tile([128, JTC, HG * VSTR], BF16, bufs=1, name=f"v{s}")
                   for s in range(NSETS)]
            g_ot = [[sb.tile([128, NQ], BF16, bufs=1, name=f"ot{s}{i}")
                     for i in range(2)] for s in range(NSETS)]
            for s in range(NSETS):
                nc.sync.dma_start(
                    out=g_v[s].rearrange("p t (h e) -> p t h e", e=VSTR)[:, :, :, 64:VSTR],
                    in_=vones.rearrange("p (t h e) -> p t h e", h=HG, e=4),
                )

            with tc.tile_pool(name="ps", bufs=1, space="PSUM") as ps:

                def _qt_mm(s, ib2, qx_t, cb):
                    qt_ps = ps.tile([128, 512], F32, tag="mm", bufs=2, name="qt_ps")
                    for k in range(KT):
                        nc.tensor.matmul(
                            qt_ps,
                            wq_sb[:, k, cb * 128:(cb + 1) * 128],
                            qx_t[k],
                            start=(k == 0),
                            stop=(k == KT - 1),
                        )
                    nc.vector.tensor_copy(
                        g_qt[s][cb][:, ib2 * 512:(ib2 + 1) * 512], qt_ps
                    )

                def _kproj_mm(s, j0, bw, ct_t, cb):
                    kt_ps = ps.tile([128, 512], F32, tag="mm", bufs=2, name="kt_ps")
                    for k in range(KT):
                        nc.tensor.matmul(
                            kt_ps[:, 0:bw],
                            wk_sb[:, k, cb * 128:(cb + 1) * 128],
                            ct_t[k][:, 0:bw],
                            start=(k == 0),
                            stop=(k == KT - 1),
                        )
                    nc.vector.tensor_copy(g_kt[s][cb][:, j0:j0 + bw], kt_ps[:, 0:bw])

                def _vproj_mm(s, j0, ct_t, js):
                    # tag "mm", not "pv": when run as filler inside the
                    # attention stream both pv bufs are held by the open
                    # PV accumulators (tag-pv alloc would deadlock)
                    v_ps = ps.tile([128, CG], F32, tag="mm", bufs=2, name="v_ps")
                    for k in range(KT):
                        nc.tensor.matmul(
                            v_ps,
                            ct_t[k][:, js * 128:(js + 1) * 128],
                            wv_sb[:, k, :],
                            start=(k == 0),
                            stop=(k == KT - 1),
                        )
                    nc.vector.tensor_copy(
                        g_v[s][:, j0 // 128 + js].rearrange(
                            "p (h e) -> p h e", e=VSTR
                        )[:, :, 0:64],
                        v_ps.rearrange("p (h e) -> p h e", e=64),
                    )

                def _phase1_inline(s, qx0=None, ct_t=None, lean=False):
                    """lean=True emits only what the stream head needs (qt/kt
                    cb0 + v js0/js1) and returns the rest as block-0 fillers,
                    shortening the single-shot startup critical path."""
                    j0, bw = jblocks[0]
                    if qx0 is None:
                        qx0 = _qt_dma(0)
                        ct_t = _ct_dma(j0, bw)
                    if lean:
                        _qt_mm(s, 0, qx0, 0)
                        _kproj_mm(s, j0, bw, ct_t, 0)
                        _vproj_mm(s, j0, ct_t, 0)
                        _vproj_mm(s, j0, ct_t, 1)
                        # cb1 data is first needed when block 1's sts are
                        # prefetched (stream idx 11), so emit late enough
                        # not to starve block 0's own st/pv stream
                        return [
                            (1, lambda: _vproj_mm(s, j0, ct_t, 2)),
                            (2, lambda: _vproj_mm(s, j0, ct_t, 3)),
                            (5, lambda: _qt_mm(s, 0, qx0, 1)),
                            (7, lambda: _kproj_mm(s, j0, bw, ct_t, 1)),
                        ]
                    _qt_mm(s, 0, qx0, 0)
                    _qt_mm(s, 0, qx0, 1)
                    for cb in range(2):
                        _kproj_mm(s, j0, bw, ct_t, cb)
                    for js in range(bw // 128):
                        _vproj_mm(s, j0, ct_t, js)
                    return []

                def _emit_st(s, hp, ib2, jt):
                    st = ps.tile([128, 1024], F32, tag="st", bufs=2, name="st")
                    for b in range(2):
                        nc.tensor.matmul(
                            st[:, b * 512:(b + 1) * 512],
                            g_kt[s][hp][b * 64:(b + 1) * 64, jt * 128:(jt + 1) * 128],
                            g_qt[s][hp][b * 64:(b + 1) * 64, ib2 * 512:(ib2 + 1) * 512],
                            start=True,
                            stop=True,
                        )
                    return st

                def _exp(st):
                    e = sb.tile([128, 1024], BF16, tag="et", bufs=4, name="e")
                    nc.scalar.activation(e, st, AF.Exp, scale=SCALE)
                    return e

                def _alloc_pvs():
                    return [
                        ps.tile([65, 512], F32, tag="pv", bufs=2, name="pv")
                        for _ in range(2)
                    ]

                def _pv_step(s, hp, jt, e, pvs):
                    for b in range(2):
                        h = 2 * hp + b
                        nc.tensor.matmul(
                            pvs[b],
                            g_v[s][:, jt, h * VSTR:h * VSTR + 65],
                            e[:, b * 512:(b + 1) * 512],
                            start=(jt == 0),
                            stop=(jt == JTC - 1),
                        )

                def _normalize(s, hp, ib2, pvs, tail=False):
                    """v2 form: pvc copy first (frees the pv PSUM bank for
                    the next block's accumulation ~2us earlier than reading
                    PSUM from the final multiply would — that ordering cost
                    ~19us/pass in sim), denominator row copied to partition
                    0 (reciprocal_approx_fast needs SBUF operands and equal
                    in/out base partitions; PSUM input NaNs on HW)."""
                    pvcs, dens = [], []
                    for b in range(2):
                        pvc = sb.tile([65, 512], F32, tag="pvc", bufs=2, name="pvc")
                        nc.vector.tensor_copy(pvc, pvs[b])
                        pvcs.append(pvc)
                    for b in range(2):
                        dr = sb.tile([1, 512], F32, tag="dr", bufs=2, name="dr")
                        nc.vector.tensor_copy(dr, pvcs[b][64:65, :])
                        rec = sb.tile([1, 512], F32, tag="rec", bufs=2, name="rec")
                        nc.vector.reciprocal_approx_fast(out=rec, in_=dr)
                        den = sb.tile([64, 512], F32, tag="den", bufs=2, name="den")
                        nc.gpsimd.partition_broadcast(den, rec[0:1, :])
                        dens.append(den)
                    for b in range(2):
                        nc.vector.tensor_mul(
                            g_ot[s][hp][b * 64:(b + 1) * 64,
                                        ib2 * 512:(ib2 + 1) * 512],
                            pvcs[b][0:64, :],
                            dens[b],
                        )

                def _oproj_unit(s, ib2, it, m, copy_act=False, ot=None):
                    ot = ot if ot is not None else g_ot[s]
                    itg = ib2 * 4 + it
                    op = ps.tile([128, 512], F32, tag="mm", bufs=2, name="op")
                    for kk in range(2):
                        nc.tensor.matmul(
                            op,
                            ot[kk][:, itg * 128:(itg + 1) * 128],
                            wo_sb[:, kk, m * 512:(m + 1) * 512],
                            start=(kk == 0),
                            stop=(kk == 1),
                        )
                    osb = sb.tile([128, 512], BF16, tag="osb", bufs=3, name="osb")
                    if copy_act:
                        nc.scalar.activation(osb, op, AF.Copy)
                    else:
                        nc.vector.tensor_copy(osb, op)
                    nc.sync.dma_start(
                        out=outp[itg * 128:(itg + 1) * 128, m * 512:(m + 1) * 512],
                        in_=osb,
                    )

                def _oproj_fillers(s, ib2, units, jt0):
                    # each unit is 2 matmuls; jt0 leaves room for the
                    # producing normalize chain to finish
                    return [
                        (jt0 + i,
                         (lambda ib2=ib2, it=it, m=m: _oproj_unit(s, ib2, it, m)))
                        for i, (it, m) in enumerate(
                            ((u // 2, u % 2) for u in units)
                        )
                    ]

                def _qtproj_fillers(s, ib2):
                    # DMA issue up front; the 2x8 matmuls from jt=6
                    box = {}

                    def dma(ib2=ib2):
                        box["qx"] = _qt_dma(ib2)

                    return [(0, dma)] + [
                        (6 + 3 * cb,
                         (lambda ib2=ib2, cb=cb: _qt_mm(s, ib2, box["qx"], cb)))
                        for cb in range(2)
                    ]

                def _kv_fillers(s, blocks):
                    """K/V projection for jblocks[1:], interleaved into the
                    first attention block: block k (j-tiles 4k..4k+3) is
                    emitted by jt 4(k-1) so the ST prefetch (jt+2) never
                    outruns the kt/v writes."""
                    out = []
                    for k, (j0, bw) in blocks:
                        base = 4 * (k - 1)
                        box = {}

                        def dma(j0=j0, bw=bw, box=box):
                            box["ct"] = _ct_dma(j0, bw)

                        out.append((base, dma))
                        for cb in range(2):
                            out.append(
                                (base + cb, lambda j0=j0, bw=bw, cb=cb, box=box:
                                 _kproj_mm(s, j0, bw, box["ct"], cb))
                            )
                        for js in range(bw // 128):
                            out.append(
                                (base + 2 + js // 2, lambda j0=j0, js=js, box=box:
                                 _vproj_mm(s, j0, box["ct"], js))
                            )
                    return out

                def _phase1_fillers(s):
                    """Pass k's tail fillers computing pass k+1's (set s)
                    block-0 data. Returned as {block_idx: [(jt, fn), ...]}
                    for the last four attention blocks (idx 4..7)."""
                    j0, bw = jblocks[0]
                    box = {}

                    def qdma():
                        box["qx"] = _qt_dma(0)

                    def cdma(j0=j0, bw=bw):
                        box["ct"] = _ct_dma(j0, bw)

                    return {
                        3: [(9, qdma), (10, cdma)],
                        4: [(6, lambda: _qt_mm(s, 0, box["qx"], 0)),
                            (9, lambda: _qt_mm(s, 0, box["qx"], 1))],
                        5: [(6, lambda: _kproj_mm(s, j0, bw, box["ct"], 0)),
                            (9, lambda: _kproj_mm(s, j0, bw, box["ct"], 1))],
                        6: [(2 + 3 * js, lambda js=js: _vproj_mm(s, j0, box["ct"], js))
                            for js in range(bw // 128)],
                    }

                def _one_pass(s, defer_in=None, prep_next=None,
                              tail_normalize=False, extra_f0=()):
                    """One full pass using activation-tile set s.

                    defer_in: the OTHER set's ot pair -- emit its final-block
                    oproj units here (their data has been ready since the
                    previous pass), and skip our own final-block oproj.
                    prep_next: set index to run phase-1 for, as tail fillers.
                    """
                    # per-block filler queues, keyed by block index in the
                    # global stream order [(hp0,ib0),(hp1,ib0),(hp0,ib1),...]
                    fq = {bi: [] for bi in range(2 * IB)}
                    fq[0] = sorted(
                        list(extra_f0) + _kv_fillers(s, list(enumerate(jblocks))[1:]),
                        key=lambda x: x[0])
                    for ib2 in range(1, IB):
                        fq[2 * ib2] = _oproj_fillers(s, ib2 - 1, range(0, 6), 4)
                    for ib2 in range(0, IB - 1):
                        fl = list(_qtproj_fillers(s, ib2 + 1))
                        if ib2 >= 1:
                            fl += _oproj_fillers(s, ib2 - 1, range(6, 8), 2)
                        fl.sort(key=lambda x: x[0])
                        fq[2 * ib2 + 1] = fl
                    fq[2 * IB - 1] = _oproj_fillers(s, IB - 2, range(6, 8), 2)
                    if defer_in is not None:
                        dunits = [
                            (lambda u=u: _oproj_unit(
                                s, IB - 1, u // 2, u % 2, ot=defer_in))
                            for u in range(8)
                        ]
                        fq[2 * IB - 2] = sorted(
                            fq[2 * IB - 2] + list(enumerate(dunits[:4])),
                            key=lambda x: x[0])
                        fq[2 * IB - 1] = sorted(
                            fq[2 * IB - 1] + list(enumerate(dunits[4:])),
                            key=lambda x: x[0])
                    if prep_next is not None:
                        for bi, fl in _phase1_fillers(prep_next).items():
                            fq[bi] = sorted(fq[bi] + fl, key=lambda x: x[0])

                    # global attention stream: depth-2 st prefetch crossing
                    # block boundaries so exp never waits at a transition
                    steps = [(2 * ib2 + hp, hp, ib2, jt)
                             for ib2 in range(IB) for hp in range(2)
                             for jt in range(JTC)]
                    st_q = {}
                    for i in range(2):
                        _, hp, ib2, jt = steps[i]
                        st_q[i] = _emit_st(s, hp, ib2, jt)
                    pvs_by_block = {}
                    for idx, (bi, hp, ib2, jt) in enumerate(steps):
                        if jt == 0:
                            pvs_by_block[bi] = _alloc_pvs()
                        e = _exp(st_q.pop(idx))
                        if idx + 2 < len(steps):
                            _, hp2, ib22, jt2 = steps[idx + 2]
                            st_q[idx + 2] = _emit_st(s, hp2, ib22, jt2)
                        _pv_step(s, hp, jt, e, pvs_by_block[bi])
                        q = fq[bi]
                        while q and q[0][0] <= jt:
                            q.pop(0)[1]()
                        if jt == JTC - 1:
                            _normalize(s, hp, ib2, pvs_by_block.pop(bi),
                                       tail=(tail_normalize and bi == 2 * IB - 1))
                            for _, f in fq[bi]:
                                f()
                            fq[bi] = []

                    if defer_in is None:
                        for u in range(8):
                            _oproj_unit(s, IB - 1, u // 2, u % 2,
                                        copy_act=(u % 2 == 1))

                if reps == 1:
                    f0 = _phase1_inline(0, p1_qx, p1_ct,
                                        lean=bool(int(os.environ.get(
                                            "BASS_ATTN_LEAN", "0"))))
                    _one_pass(0, tail_normalize=True, extra_f0=f0)
                elif pipe:
                    # two unrolled passes per iteration with alternating
                    # buffer sets; each pass emits the OTHER set's final
                    # oproj (previous pass's data; garbage on iteration 0,
                    # harmless for timing-only loops) and the other set's
                    # phase-1 projections for the NEXT pass.
                    assert reps % 2 == 0, "pipelined builds need even reps"
                    _phase1_inline(0, p1_qx, p1_ct)
                    with tc.For_i(0, reps // 2, 1):
                        _one_pass(0, defer_in=g_ot[1], prep_next=1)
                        _one_pass(1, defer_in=g_ot[0], prep_next=0)
                else:
                    with tc.For_i(0, reps, 1):
                        _phase1_inline(0)
                        _one_pass(0, tail_normalize=True)

    nc.compile()
    return nc


def _nkc_for_mask(mask):
    """Compacted key count: max unmasked keys over batches, rounded to 128."""
    counts = [int((~mask[bi]).sum()) for bi in range(mask.shape[0])]
    nkc = max(max(counts), 1)
    nkc = min(((nkc + 127) // 128) * 128, NK)
    return nkc


def _bf16(a):
    import ml_dtypes

    return np.ascontiguousarray(a).astype(ml_dtypes.bfloat16)


def _prep_core_inputs(q, context, mask, Wq, Wkv, Wout, core, nkc=NK):
    bi, g = core // 4, core % 4
    c0 = g * CG
    JTC = nkc // 128
    keep_idx = np.nonzero(~mask[bi])[0]
    ctx_c = np.zeros((nkc, D), dtype=np.float32)
    ctx_c[: len(keep_idx)] = context[bi][keep_idx]
    # per-key keep flag in the V ones-column: padding keys have K=V=0 so
    # their exp(0)=1 weight contributes 0 to both numerator and denominator
    keep = np.zeros(nkc, dtype=np.float32)
    keep[: len(keep_idx)] = 1.0
    km = np.ascontiguousarray(keep.reshape(JTC, 128).T)       # [128, JTC]
    vo = np.zeros((128, JTC, HG, 4), dtype=np.float32)
    vo[:, :, :, 0] = km[:, :, None]
    return {
        "qT": _bf16(q[bi].T),
        "cT": _bf16(ctx_c.T),
        "wq": _bf16(Wq[:, c0:c0 + CG]),
        "wk": _bf16(Wkv[:, c0:c0 + CG]),
        "wv": _bf16(Wkv[:, D + c0:D + c0 + CG]),
        "wo": _bf16(Wout[c0:c0 + CG, :]),
        "vones": _bf16(vo.reshape(128, -1)),
    }


def kernel(q, context, mask, Wq, Wkv, Wout, b_out):
    from concourse.bass_utils import run_bass_kernel_spmd

    q = np.asarray(q, dtype=np.float32)
    context = np.asarray(context, dtype=np.float32)
    mask = np.asarray(mask)
    Wq = np.asarray(Wq, dtype=np.float32)
    Wkv = np.asarray(Wkv, dtype=np.float32)
    Wout = np.asarray(Wout, dtype=np.float32)
    b_out = np.asarray(b_out, dtype=np.float32)

    nkc = _nkc_for_mask(mask)
    key = ("nc", nkc)
    if key not in _CACHE:
        _CACHE[key] = build_nc(nkc=nkc)
    nc = _CACHE[key]
    _CACHE["nc"] = nc
    _CACHE["nkc"] = nkc

    in_maps = [
        _prep_core_inputs(q, context, mask, Wq, Wkv, Wout, c, nkc=nkc)
        for c in range(N_CORES)
    ]

    trace = bool(int(os.environ.get("BASS_ATTN_TRACE", "0")))
    res = run_bass_kernel_spmd(nc, in_maps, list(range(N_CORES)), trace=trace)
    _CACHE["last_results"] = res
    _CACHE["last_in_maps"] = in_maps

    out = np.empty((B, NQ, D), dtype=np.float32)
    for bi in range(B):
        acc = res.results[4 * bi]["outp"].astype(np.float32).copy()
        for g in range(1, 4):
            acc += res.results[4 * bi + g]["outp"].astype(np.float32)
        out[bi] = acc + b_out[None, :]
    return out
